# revision 1
# baseline (speedup 1.0000x reference)
"""ConvCaps dynamic-routing kernel for 8 TRN2 NeuronCores.

Strategy (data-parallel over batch B=8, one batch element per core):
  - Grouped 3x3 conv (groups=D=32) done as one matmul per group per
    pixel-tile: stationary = im2col patches [72, npx], moving = weights
    [72, 512], PSUM out [npx, 512] -> u tile in SBUF laid out
    [px_partition, D, c, d].  No u traffic to DRAM at all.
  - 3 dynamic-routing iterations run on the Vector engine entirely
    in SBUF with px on partitions: softmax over d, s/a einsums as
    multiply + segmented tensor_reduce over D (resp. c).
  - Output s [px, (c,d)] is PE-transposed to [(c,d), px] and DMA'd out.
"""

import numpy as np
from contextlib import ExitStack

import concourse.bacc as bacc
import concourse.bass as bass
import concourse.tile as tile
import concourse.mybir as mybir
from concourse.bass_utils import run_bass_kernel_spmd
from concourse.masks import make_identity

F32 = mybir.dt.float32
AF = mybir.ActivationFunctionType

B = 8
C_IN, D_IN = 8, 32
C_OUT, D_OUT = 16, 32
KS = 3
H = W = 32
HO = WO = 30
NPX = HO * WO                 # 900 output pixels per batch element
KDIM = C_IN * KS * KS         # 72 = contraction dim of the conv matmul
CD = C_OUT * D_OUT            # 512 out-channels per group
ITERS = 3
P = 128
EPS = 1e-8
# pixel tiles = groups of output rows (30 px each); partition dim <= 128
ROW_TILES = [(0, 4), (4, 4), (8, 4), (12, 4), (16, 4), (20, 4), (24, 4), (28, 2)]
DCH = 8                       # D-chunk size for the einsum passes
NCH = D_IN // DCH


def _body(ctx, tc, xb, wt, b0, out, zero_prior):
    nc = tc.nc
    consts = ctx.enter_context(tc.tile_pool(name="consts", bufs=1))
    wpool = ctx.enter_context(tc.tile_pool(name="wpool", bufs=1))
    x9pool = ctx.enter_context(tc.tile_pool(name="x9pool", bufs=1))
    upool = ctx.enter_context(tc.tile_pool(name="upool", bufs=1))
    rpool = ctx.enter_context(tc.tile_pool(name="rpool", bufs=1))
    tmppool = ctx.enter_context(tc.tile_pool(name="tmppool", bufs=2))
    opool = ctx.enter_context(tc.tile_pool(name="opool", bufs=2))
    psum_c = ctx.enter_context(tc.tile_pool(name="psum_c", bufs=6, space="PSUM"))
    psum_t = ctx.enter_context(tc.tile_pool(name="psum_t", bufs=2, space="PSUM"))

    w_sb = wpool.tile([KDIM, D_IN * CD], F32)
    nc.sync.dma_start(w_sb[:], wt)
    ident = consts.tile([P, P], F32)
    make_identity(nc, ident)
    b0_sb = consts.tile([P, D_IN, D_OUT], F32)
    nc.sync.dma_start(b0_sb[:], b0)

    for (r0, nr) in ROW_TILES:
        npx = nr * WO
        pxs = slice(0, npx)

        # ---- im2col: 9 shifted window loads; partition k = (kh*3+kw)*8 + C
        x9 = x9pool.tile([KDIM, D_IN, 4, WO], F32, tag="x9")
        for kh in range(KS):
            for kw in range(KS):
                kk = kh * KS + kw
                for j in range(nr):
                    # per-row copy keeps both DMA access patterns <= 3 dims
                    nc.sync.dma_start(
                        x9[kk * C_IN:(kk + 1) * C_IN, :, j, :],
                        xb[:, :, r0 + kh + j, kw:kw + WO],
                    )

        # ---- grouped conv: one matmul per group, psum -> u_t on ScalarE
        # u_t layout (D, c, d); strided reduces measure faster than dense
        u_t = upool.tile([P, D_IN, C_OUT, D_OUT], F32, tag="u")
        for g in range(D_IN):
            pu = psum_c.tile([P, CD], F32, tag="pu")
            nc.tensor.matmul(
                pu[pxs, :],
                x9[:, g, 0:nr, :],
                w_sb[:, g * CD:(g + 1) * CD],
                start=True, stop=True,
            )
            nc.scalar.copy(u_t[pxs, g], pu[pxs, :])

        # ---- routing state tiles
        b_t = rpool.tile([P, D_IN, D_OUT], F32, tag="b")
        c_t = rpool.tile([P, D_IN, D_OUT], F32, tag="c")
        s_t = rpool.tile([P, C_OUT, D_OUT], F32, tag="s")
        sk_t = rpool.tile([P, C_OUT, D_OUT], F32, tag="sk")
        sq_t = rpool.tile([P, C_OUT, D_OUT], F32, tag="sq")
        v_t = rpool.tile([P, C_OUT, D_OUT], F32, tag="v")
        ak_t = rpool.tile([P, DCH, D_OUT], F32, tag="ak")
        n2_t = rpool.tile([P, D_OUT], F32, tag="n2")
        r_t = rpool.tile([P, D_OUT], F32, tag="r")
        f_t = rpool.tile([P, D_OUT], F32, tag="f")
        ssum = rpool.tile([P, D_IN], F32, tag="ssum")

        nc.scalar.copy(b_t[pxs], b0_sb[pxs])

        for it in range(ITERS):
            first = it == 0
            last = it == ITERS - 1
            uniform0 = first and zero_prior

            # softmax over d (no max-subtraction: logits are O(1) here)
            if not uniform0:
                nc.scalar.activation(c_t[pxs], b_t[pxs], AF.Exp)
                nc.vector.reduce_sum(ssum[pxs], c_t[pxs],
                                     axis=mybir.AxisListType.X)
                nc.vector.reciprocal(ssum[pxs], ssum[pxs])
                nc.vector.tensor_mul(
                    c_t[pxs], c_t[pxs],
                    ssum[pxs].unsqueeze(2).broadcast_to((npx, D_IN, D_OUT)))

            # s[c,d] = sum_D c[D,d] * u[D,c,d]   (chunked over D;
            # multiplies on GpSimd, segmented reduces on Vector)
            if uniform0:
                # c is uniform 1/32: one big reduce over all of D
                red_in = u_t[pxs].rearrange("p a b c -> p (b c) a")
                nc.vector.reduce_sum(s_t[pxs], red_in,
                                     axis=mybir.AxisListType.X)
                nc.vector.tensor_scalar_mul(s_t[pxs], s_t[pxs], 1.0 / D_IN)
            else:
                for k in range(NCH):
                    dk = slice(k * DCH, (k + 1) * DCH)
                    dst = s_t if k == 0 else sk_t
                    tmp = tmppool.tile([P, DCH, C_OUT, D_OUT], F32, tag="tmp")
                    nc.gpsimd.tensor_mul(
                        tmp[pxs], u_t[pxs, dk],
                        c_t[pxs, dk].unsqueeze(2)
                        .broadcast_to((npx, DCH, C_OUT, D_OUT)))
                    red_in = tmp[pxs].rearrange("p a b c -> p (b c) a")
                    nc.vector.reduce_sum(dst[pxs], red_in,
                                         axis=mybir.AxisListType.X)
                    if k > 0:
                        nc.vector.tensor_add(s_t[pxs], s_t[pxs], sk_t[pxs])

            if last:
                break

            # squash over c: v = s * n2 / ((1+n2) * sqrt(n2+eps))
            nc.scalar.square(sq_t[pxs], s_t[pxs])
            nc.vector.reduce_sum(n2_t[pxs], sq_t[pxs].transpose([0, 2, 1]),
                                 axis=mybir.AxisListType.X)
            nc.vector.tensor_scalar_add(r_t[pxs], n2_t[pxs], EPS)
            nc.scalar.activation(r_t[pxs], r_t[pxs], AF.Sqrt)
            nc.vector.tensor_scalar_add(f_t[pxs], n2_t[pxs], 1.0)
            nc.vector.tensor_mul(f_t[pxs], f_t[pxs], r_t[pxs])
            nc.vector.reciprocal(f_t[pxs], f_t[pxs])
            nc.vector.tensor_mul(f_t[pxs], f_t[pxs], n2_t[pxs])
            nc.vector.tensor_mul(
                v_t[pxs], s_t[pxs],
                f_t[pxs].unsqueeze(1).broadcast_to((npx, C_OUT, D_OUT)))

            # b[D,d] += sum_c u[D,c,d] * v[c,d]   (chunked over D)
            for k in range(NCH):
                dk = slice(k * DCH, (k + 1) * DCH)
                tmp = tmppool.tile([P, DCH, C_OUT, D_OUT], F32, tag="tmp")
                nc.gpsimd.tensor_mul(
                    tmp[pxs], u_t[pxs, dk],
                    v_t[pxs].unsqueeze(1)
                    .broadcast_to((npx, DCH, C_OUT, D_OUT)))
                nc.vector.reduce_sum(ak_t[pxs],
                                     tmp[pxs].transpose([0, 1, 3, 2]),
                                     axis=mybir.AxisListType.X)
                nc.vector.tensor_add(b_t[pxs, dk], b_t[pxs, dk], ak_t[pxs])

        # ---- write s out as [(c,d), px]: PE transpose in 128-row blocks
        s_flat = s_t[:].rearrange("p a b -> p (a b)")
        for blk in range(CD // P):
            pt = psum_t.tile([P, 120], F32, tag="pt")
            nc.tensor.transpose(
                pt[:, pxs], s_flat[pxs, blk * P:(blk + 1) * P],
                ident[pxs, pxs])
            ob = opool.tile([P, 120], F32, tag="ob")
            nc.scalar.copy(ob[:, pxs], pt[:, pxs])
            nc.sync.dma_start(
                out[blk * P:(blk + 1) * P, r0 * WO:r0 * WO + npx],
                ob[:, pxs])


_CACHE = {}


def _build(zero_prior: bool):
    key = ("v3", zero_prior)
    if key in _CACHE:
        return _CACHE[key]
    nc = bacc.Bacc("TRN2", target_bir_lowering=False, debug=False,
                   enable_asserts=True, num_devices=B)
    xb = nc.dram_tensor("xb", [C_IN, D_IN, H, W], F32,
                        kind="ExternalInput").ap()
    wt = nc.dram_tensor("wt", [KDIM, D_IN * CD], F32,
                        kind="ExternalInput").ap()
    b0 = nc.dram_tensor("b0", [P, D_IN, D_OUT], F32,
                        kind="ExternalInput").ap()
    out = nc.dram_tensor("out", [CD, NPX], F32, kind="ExternalOutput").ap()
    with tile.TileContext(nc) as tc:
        with ExitStack() as ctx:
            _body(ctx, tc, xb, wt, b0, out, zero_prior)
    nc.compile()
    _CACHE[key] = nc
    return nc


def _prep_inputs(x, conv_w, prior):
    # weights: rows (D,c,d) x (C,kh,kw) -> [k=(kh,kw,C), (D,c,d)]
    wt = conv_w.reshape(D_IN, C_OUT, D_OUT, C_IN, KS, KS)
    wt = np.ascontiguousarray(wt.transpose(4, 5, 3, 0, 1, 2)).reshape(KDIM, D_IN * CD)
    pb = np.broadcast_to(prior.reshape(D_IN, D_OUT), (P, D_IN, D_OUT))
    b0 = np.ascontiguousarray(pb).astype(np.float32)
    in_maps = [
        {"xb": np.ascontiguousarray(x[b]), "wt": wt, "b0": b0}
        for b in range(B)
    ]
    return in_maps


def kernel(x, conv_w, prior):
    x = np.asarray(x, dtype=np.float32)
    conv_w = np.asarray(conv_w, dtype=np.float32)
    prior = np.asarray(prior, dtype=np.float32)
    zero_prior = not np.any(prior)
    nc = _build(zero_prior)
    in_maps = _prep_inputs(x, conv_w, prior)
    res = run_bass_kernel_spmd(nc, in_maps, list(range(B)))
    outs = [res.results[b]["out"].reshape(C_OUT, D_OUT, HO, WO)
            for b in range(B)]
    return np.stack(outs, axis=0).astype(np.float32)



# revision 5
# speedup vs baseline: 2.7996x; 2.7996x over previous
"""ConvCaps dynamic-routing kernel for 8 TRN2 NeuronCores.

Strategy (data-parallel over batch B=8, one batch element per core):
  - Everything in bf16 (tolerance is 2e-2; bf16 lands ~1e-3).
  - Grouped 3x3 conv (groups=D=32) as one bf16 matmul per group per
    pixel tile: stationary = im2col patches [72, npx], moving = weights
    [72, 512], PSUM fp32 -> u tile in SBUF as bf16 [px, D, c, d].
  - iter-0 s (zero prior => uniform c) comes free from TensorE: a second
    moving pass per group accumulates sum_D u into one PSUM bank.
  - Routing einsum contractions run on the Vector engine as bf16
    tensor_tensor multiplies (2x mode) + in-place binary-tree adds
    (2x mode) instead of TENSOR_REDUCE (1x, ~1.6 cyc/elem measured).
  - Output s [px, (c,d)] is PE-transposed to [(c,d), px] and DMA'd out.
"""

import numpy as np
from contextlib import ExitStack

import ml_dtypes

import concourse.bacc as bacc
import concourse.bass as bass
import concourse.tile as tile
import concourse.mybir as mybir
from concourse.bass_utils import run_bass_kernel_spmd
from concourse.masks import make_identity

F32 = mybir.dt.float32
BF16 = mybir.dt.bfloat16
AF = mybir.ActivationFunctionType

B = 8
C_IN, D_IN = 8, 32
C_OUT, D_OUT = 16, 32
KS = 3
H = W = 32
HO = WO = 30
NPX = HO * WO                 # 900 output pixels per batch element
KDIM = C_IN * KS * KS         # 72 = contraction dim of the conv matmul
CD = C_OUT * D_OUT            # 512 out-channels per group
ITERS = 3
P = 128
EPS = 1e-8
ROW_TILES = [(0, 4), (4, 4), (8, 4), (12, 4), (16, 4), (20, 4), (24, 4), (28, 2)]


def _tree_reduce_flat(nc, tmp, pxs, n, dst):
    """In-place binary tree sum of tmp[pxs, :n] down to dst[pxs] (n//? -> 512).

    tmp is a [P, n] view; halves n each level with bf16 tensor_tensor adds
    (2x DVE mode) until 1024, then the final add writes into dst (512 wide).
    """
    while n > 1024:
        h = n // 2
        nc.vector.tensor_add(tmp[pxs, 0:h], tmp[pxs, 0:h], tmp[pxs, h:n])
        n = h
    nc.vector.tensor_add(dst, tmp[pxs, 0:512], tmp[pxs, 512:1024])


def _tree_reduce_c(nc, tmp, pxs, dst):
    """Sum tmp[pxs, D, c, d] over c (16) -> dst [pxs, D, d] (in-place tree)."""
    c = C_OUT
    while c > 2:
        h = c // 2
        nc.vector.tensor_add(tmp[pxs, :, 0:h, :], tmp[pxs, :, 0:h, :],
                             tmp[pxs, :, h:c, :])
        c = h
    nc.vector.tensor_add(dst, tmp[pxs, :, 0, :], tmp[pxs, :, 1, :])


def _body(ctx, tc, xb, wt, b0, out, zero_prior):
    nc = tc.nc
    consts = ctx.enter_context(tc.tile_pool(name="consts", bufs=1))
    wpool = ctx.enter_context(tc.tile_pool(name="wpool", bufs=1))
    x9pool = ctx.enter_context(tc.tile_pool(name="x9pool", bufs=2))
    upool = ctx.enter_context(tc.tile_pool(name="upool", bufs=2))
    s0pool = ctx.enter_context(tc.tile_pool(name="s0pool", bufs=2))
    rpool = ctx.enter_context(tc.tile_pool(name="rpool", bufs=1))
    tmppool = ctx.enter_context(tc.tile_pool(name="tmppool", bufs=1))
    opool = ctx.enter_context(tc.tile_pool(name="opool", bufs=2))
    psum_c = ctx.enter_context(tc.tile_pool(name="psum_c", bufs=4, space="PSUM"))
    psum_s = ctx.enter_context(tc.tile_pool(name="psum_s", bufs=2, space="PSUM"))
    psum_t = ctx.enter_context(tc.tile_pool(name="psum_t", bufs=2, space="PSUM"))

    w_sb = wpool.tile([KDIM, D_IN * CD], BF16)
    nc.sync.dma_start(w_sb[:], wt)
    ident = consts.tile([P, P], BF16)
    make_identity(nc, ident)
    for cval in (EPS, 1.0):
        cb = consts.tile([P, 1], F32, tag=f"const_{cval}")
        nc.gpsimd.memset(cb[:], cval)
        nc.const_aps.aps[(F32, cval)] = cb[:]
    if not zero_prior:
        b0_sb = consts.tile([P, D_IN, D_OUT], BF16)
        nc.sync.dma_start(b0_sb[:], b0)

    for (r0, nr) in ROW_TILES:
        npx = nr * WO
        pxs = slice(0, npx)

        # ---- im2col: 9 shifted window loads; partition k = (kh*3+kw)*8 + C
        x9 = x9pool.tile([KDIM, D_IN, 4, WO], BF16, tag="x9")
        for kh in range(KS):
            for kw in range(KS):
                kk = kh * KS + kw
                for j in range(nr):
                    nc.sync.dma_start(
                        x9[kk * C_IN:(kk + 1) * C_IN, :, j, :],
                        xb[:, :, r0 + kh + j, kw:kw + WO],
                    )

        # ---- grouped conv: one bf16 matmul per group; a second moving pass
        # accumulates sum_D u into ps0 (free iter-0 s when prior is zero).
        u_t = upool.tile([P, D_IN, C_OUT, D_OUT], BF16, tag="u")
        s0_sb = s0pool.tile([P, C_OUT, D_OUT], BF16, tag="s0")
        if zero_prior:
            ps0 = psum_s.tile([P, CD], F32, tag="ps0")
        for g in range(D_IN):
            pu = psum_c.tile([P, CD], F32, tag="pu")
            nc.tensor.matmul(
                pu[pxs, :],
                x9[:, g, 0:nr, :],
                w_sb[:, g * CD:(g + 1) * CD],
                start=True, stop=True,
            )
            if zero_prior:
                nc.tensor.matmul(
                    ps0[pxs, :],
                    x9[:, g, 0:nr, :],
                    w_sb[:, g * CD:(g + 1) * CD],
                    start=(g == 0), stop=(g == D_IN - 1),
                    skip_group_check=True,
                )
            nc.scalar.copy(u_t[pxs, g], pu[pxs, :])
        if zero_prior:
            nc.scalar.activation(s0_sb[pxs], ps0[pxs, :], AF.Copy,
                                 scale=1.0 / D_IN)

        # ---- routing state tiles
        b_t = rpool.tile([P, D_IN, D_OUT], BF16, tag="b")
        c_e = rpool.tile([P, D_IN, D_OUT], BF16, tag="ce")
        c_t = rpool.tile([P, D_IN, D_OUT], BF16, tag="c")
        ak_t = rpool.tile([P, D_IN, D_OUT], BF16, tag="ak")
        s_t = rpool.tile([P, C_OUT, D_OUT], BF16, tag="s")
        sq_t = rpool.tile([P, C_OUT, D_OUT], F32, tag="sq")
        v_t = rpool.tile([P, C_OUT, D_OUT], BF16, tag="v")
        n2_t = rpool.tile([P, D_OUT], F32, tag="n2")
        r_t = rpool.tile([P, D_OUT], F32, tag="r")
        q_t = rpool.tile([P, D_OUT], F32, tag="q")
        f_t = rpool.tile([P, D_OUT], F32, tag="f")
        rsum = rpool.tile([P, D_IN], F32, tag="rsum")
        tmp = tmppool.tile([P, D_IN, C_OUT, D_OUT], BF16, tag="tmp")
        tmp_flat = tmp[:].rearrange("p a b c -> p (a b c)")

        if not zero_prior:
            nc.scalar.copy(b_t[pxs], b0_sb[pxs])

        for it in range(ITERS):
            first = it == 0
            last = it == ITERS - 1
            uniform0 = first and zero_prior

            # softmax over d (no max-subtraction: logits are O(1) here)
            if not uniform0:
                src = b0_sb if (first and not zero_prior) else b_t
                nc.scalar.activation(c_e[pxs], src[pxs], AF.Exp)
                nc.vector.reduce_sum(rsum[pxs], c_e[pxs],
                                     axis=mybir.AxisListType.X)
                nc.vector.reciprocal(rsum[pxs], rsum[pxs])
                nc.vector.tensor_mul(
                    c_t[pxs], c_e[pxs],
                    rsum[pxs].unsqueeze(2).broadcast_to((npx, D_IN, D_OUT)))

            # s[c,d] = sum_D c[D,d] * u[D,c,d]
            if uniform0:
                s_cur = s0_sb      # from the TensorE accumulation pass
            else:
                s_cur = s_t
                nc.vector.tensor_mul(
                    tmp[pxs], u_t[pxs],
                    c_t[pxs].unsqueeze(2)
                    .broadcast_to((npx, D_IN, C_OUT, D_OUT)))
                _tree_reduce_flat(nc, tmp_flat, pxs, D_IN * CD, s_t[pxs])

            if last:
                break

            # squash over c: v = s * n2 / ((1+n2) * sqrt(n2+eps))
            nc.scalar.square(sq_t[pxs], s_cur[pxs])
            nc.vector.reduce_sum(n2_t[pxs], sq_t[pxs].transpose([0, 2, 1]),
                                 axis=mybir.AxisListType.X)
            nc.scalar.activation(r_t[pxs], n2_t[pxs], AF.Sqrt, bias=EPS)
            nc.scalar.add(q_t[pxs], n2_t[pxs], 1.0)
            nc.vector.tensor_mul(f_t[pxs], q_t[pxs], r_t[pxs])
            nc.vector.reciprocal(f_t[pxs], f_t[pxs])
            nc.vector.tensor_mul(f_t[pxs], f_t[pxs], n2_t[pxs])
            nc.vector.tensor_mul(
                v_t[pxs], s_cur[pxs],
                f_t[pxs].unsqueeze(1).broadcast_to((npx, C_OUT, D_OUT)))

            # b[D,d] += sum_c u[D,c,d] * v[c,d]
            nc.vector.tensor_mul(
                tmp[pxs], u_t[pxs],
                v_t[pxs].unsqueeze(1)
                .broadcast_to((npx, D_IN, C_OUT, D_OUT)))
            if uniform0:
                # b was zero: write the reduction straight into b
                _tree_reduce_c(nc, tmp, pxs, b_t[pxs])
            else:
                _tree_reduce_c(nc, tmp, pxs, ak_t[pxs])
                nc.vector.tensor_add(b_t[pxs], b_t[pxs], ak_t[pxs])

        # ---- write s out as [(c,d), px]: PE transpose in 128-row blocks
        s_flat = s_t[:].rearrange("p a b -> p (a b)")
        for blk in range(CD // P):
            pt = psum_t.tile([P, 120], BF16, tag="pt")
            nc.tensor.transpose(
                pt[:, pxs], s_flat[pxs, blk * P:(blk + 1) * P],
                ident[pxs, pxs])
            ob = opool.tile([P, 120], F32, tag="ob")
            nc.scalar.copy(ob[:, pxs], pt[:, pxs])
            nc.sync.dma_start(
                out[blk * P:(blk + 1) * P, r0 * WO:r0 * WO + npx],
                ob[:, pxs])


_CACHE = {}


def _build(zero_prior: bool):
    key = ("v4", zero_prior)
    if key in _CACHE:
        return _CACHE[key]
    nc = bacc.Bacc("TRN2", target_bir_lowering=False, debug=False,
                   enable_asserts=True, num_devices=B)
    xb = nc.dram_tensor("xb", [C_IN, D_IN, H, W], BF16,
                        kind="ExternalInput").ap()
    wt = nc.dram_tensor("wt", [KDIM, D_IN * CD], BF16,
                        kind="ExternalInput").ap()
    b0 = nc.dram_tensor("b0", [P, D_IN, D_OUT], BF16,
                        kind="ExternalInput").ap()
    out = nc.dram_tensor("out", [CD, NPX], F32, kind="ExternalOutput").ap()
    with tile.TileContext(nc) as tc:
        with ExitStack() as ctx:
            _body(ctx, tc, xb, wt, b0, out, zero_prior)
    nc.compile()
    _CACHE[key] = nc
    return nc


def _prep_inputs(x, conv_w, prior):
    # weights: rows (D,c,d) x (C,kh,kw) -> [k=(kh,kw,C), (D,c,d)]
    wt = conv_w.reshape(D_IN, C_OUT, D_OUT, C_IN, KS, KS)
    wt = np.ascontiguousarray(wt.transpose(4, 5, 3, 0, 1, 2)).reshape(KDIM, D_IN * CD)
    wt = wt.astype(ml_dtypes.bfloat16)
    pb = np.broadcast_to(prior.reshape(D_IN, D_OUT), (P, D_IN, D_OUT))
    b0 = np.ascontiguousarray(pb).astype(ml_dtypes.bfloat16)
    xbf = x.astype(ml_dtypes.bfloat16)
    in_maps = [
        {"xb": np.ascontiguousarray(xbf[b]), "wt": wt, "b0": b0}
        for b in range(B)
    ]
    return in_maps


def kernel(x, conv_w, prior):
    x = np.asarray(x, dtype=np.float32)
    conv_w = np.asarray(conv_w, dtype=np.float32)
    prior = np.asarray(prior, dtype=np.float32)
    zero_prior = not np.any(prior)
    nc = _build(zero_prior)
    in_maps = _prep_inputs(x, conv_w, prior)
    res = run_bass_kernel_spmd(nc, in_maps, list(range(B)))
    outs = [res.results[b]["out"].reshape(C_OUT, D_OUT, HO, WO)
            for b in range(B)]
    return np.stack(outs, axis=0).astype(np.float32)


# revision 10
# speedup vs baseline: 2.9738x; 1.0622x over previous
"""ConvCaps dynamic-routing kernel for 8 TRN2 NeuronCores.

Strategy (data-parallel over batch B=8, one batch element per core):
  - Everything in bf16 (tolerance is 2e-2; bf16 lands ~1e-3).
  - Grouped 3x3 conv (groups=D=32) as one bf16 matmul per group per
    pixel tile: stationary = im2col patches [72, npx], moving = weights
    [72, 512], PSUM fp32 -> u tile in SBUF as bf16 [px, D, c, d].
  - iter-0 s (zero prior => uniform c) comes free from TensorE: a second
    moving pass per group accumulates sum_D u into one PSUM bank.
  - Routing einsum contractions run on the Vector engine as bf16
    tensor_tensor multiplies (2x mode) + in-place binary-tree adds
    (2x mode) instead of TENSOR_REDUCE (1x, ~1.6 cyc/elem measured).
  - Output s [px, (c,d)] is PE-transposed to [(c,d), px] and DMA'd out.
"""

import numpy as np
from contextlib import ExitStack

import ml_dtypes

import concourse.bacc as bacc
import concourse.bass as bass
import concourse.tile as tile
import concourse.mybir as mybir
from concourse.bass_utils import run_bass_kernel_spmd
from concourse.masks import make_identity

F32 = mybir.dt.float32
BF16 = mybir.dt.bfloat16
AF = mybir.ActivationFunctionType

B = 8
C_IN, D_IN = 8, 32
C_OUT, D_OUT = 16, 32
KS = 3
H = W = 32
HO = WO = 30
NPX = HO * WO                 # 900 output pixels per batch element
KDIM = C_IN * KS * KS         # 72 = contraction dim of the conv matmul
CD = C_OUT * D_OUT            # 512 out-channels per group
ITERS = 3
P = 128
EPS = 1e-8
ROW_TILES = [(0, 4), (4, 4), (8, 4), (12, 4), (16, 4), (20, 4), (24, 4), (28, 2)]


def _tree_reduce_flat(nc, tmp, pxs, n, dst):
    """In-place binary tree sum of tmp[pxs, :n] down to dst[pxs] (n//? -> 512).

    tmp is a [P, n] view; halves n each level with bf16 tensor_tensor adds
    (2x DVE mode) until 1024, then the final add writes into dst (512 wide).
    """
    while n > 1024:
        h = n // 2
        nc.vector.tensor_add(tmp[pxs, 0:h], tmp[pxs, 0:h], tmp[pxs, h:n])
        n = h
    nc.vector.tensor_add(dst, tmp[pxs, 0:512], tmp[pxs, 512:1024])


def _tree_reduce_c(nc, tmp, pxs, dst):
    """Sum tmp[pxs, D, c, d] over c (16) -> dst [pxs, D, d] (in-place tree)."""
    c = C_OUT
    while c > 2:
        h = c // 2
        nc.vector.tensor_add(tmp[pxs, :, 0:h, :], tmp[pxs, :, 0:h, :],
                             tmp[pxs, :, h:c, :])
        c = h
    nc.vector.tensor_add(dst, tmp[pxs, :, 0, :], tmp[pxs, :, 1, :])


def _body(ctx, tc, xb, wt, b0, out, zero_prior):
    nc = tc.nc
    consts = ctx.enter_context(tc.tile_pool(name="consts", bufs=1))
    wpool = ctx.enter_context(tc.tile_pool(name="wpool", bufs=1))
    x9pool = ctx.enter_context(tc.tile_pool(name="x9pool", bufs=2))
    upool = ctx.enter_context(tc.tile_pool(name="upool", bufs=2))
    s0pool = ctx.enter_context(tc.tile_pool(name="s0pool", bufs=2))
    rpool = ctx.enter_context(tc.tile_pool(name="rpool", bufs=2))
    tmppool = ctx.enter_context(tc.tile_pool(name="tmppool", bufs=2))
    opool = ctx.enter_context(tc.tile_pool(name="opool", bufs=2))
    psum_c = ctx.enter_context(tc.tile_pool(name="psum_c", bufs=4, space="PSUM"))
    psum_s = ctx.enter_context(tc.tile_pool(name="psum_s", bufs=2, space="PSUM"))
    psum_t = ctx.enter_context(tc.tile_pool(name="psum_t", bufs=2, space="PSUM"))

    w_sb = wpool.tile([KDIM, D_IN * CD], BF16)
    nc.sync.dma_start(w_sb[:], wt)
    ident = consts.tile([P, P], BF16)
    make_identity(nc, ident)
    for cval in (EPS, 1.0, 32.0):
        cb = consts.tile([P, 1], F32, tag=f"const_{cval}")
        nc.gpsimd.memset(cb[:], cval)
        nc.const_aps.aps[(F32, cval)] = cb[:]
    if not zero_prior:
        b0_sb = consts.tile([P, D_IN, D_OUT], BF16)
        nc.sync.dma_start(b0_sb[:], b0)

    for ti, (r0, nr) in enumerate(ROW_TILES):
        npx = nr * WO
        pxs = slice(0, npx)
        # tile 0 computes s0 with a vector tree instead of the extra PE
        # pass: u is on the critical path at startup, nothing overlaps it.
        mm_s0 = zero_prior and ti > 0

        # ---- im2col: 9 shifted window loads; partition k = (kh*3+kw)*8 + C
        x9 = x9pool.tile([KDIM, D_IN, 4, WO], BF16, tag="x9")
        di = 0
        for kh in range(KS):
            for kw in range(KS):
                kk = kh * KS + kw
                for j in range(nr):
                    # tile 0: split dispatch across both HWDGE queues
                    eng = nc.scalar if (ti == 0 and di % 2) else nc.sync
                    eng.dma_start(
                        x9[kk * C_IN:(kk + 1) * C_IN, :, j, :],
                        xb[:, :, r0 + kh + j, kw:kw + WO],
                    )
                    di += 1

        # ---- grouped conv: one bf16 matmul per group; a second moving pass
        # accumulates sum_D u into ps0 (free iter-0 s when prior is zero).
        u_t = upool.tile([P, D_IN, C_OUT, D_OUT], BF16, tag="u")
        s0_sb = s0pool.tile([P, C_OUT, D_OUT], BF16, tag="s0")
        if mm_s0:
            ps0 = psum_s.tile([P, CD], F32, tag="ps0")
        for g in range(D_IN):
            pu = psum_c.tile([P, CD], F32, tag="pu")
            nc.tensor.matmul(
                pu[pxs, :],
                x9[:, g, 0:nr, :],
                w_sb[:, g * CD:(g + 1) * CD],
                start=True, stop=True,
            )
            if mm_s0:
                nc.tensor.matmul(
                    ps0[pxs, :],
                    x9[:, g, 0:nr, :],
                    w_sb[:, g * CD:(g + 1) * CD],
                    start=(g == 0), stop=(g == D_IN - 1),
                    skip_group_check=True,
                )
            nc.scalar.copy(u_t[pxs, g], pu[pxs, :])
        if mm_s0:
            nc.scalar.activation(s0_sb[pxs], ps0[pxs, :], AF.Copy,
                                 scale=1.0 / D_IN)

        # ---- routing state tiles
        b_t = rpool.tile([P, D_IN, D_OUT], BF16, tag="b")
        c_e = rpool.tile([P, D_IN, D_OUT], BF16, tag="ce")
        c_t = rpool.tile([P, D_IN, D_OUT], BF16, tag="c")
        ak_t = rpool.tile([P, D_IN, D_OUT], BF16, tag="ak")
        s_t = rpool.tile([P, C_OUT, D_OUT], BF16, tag="s")
        sq_t = rpool.tile([P, C_OUT, D_OUT], F32, tag="sq")
        v_t = rpool.tile([P, C_OUT, D_OUT], BF16, tag="v")
        n2_t = rpool.tile([P, D_OUT], F32, tag="n2")
        r_t = rpool.tile([P, D_OUT], F32, tag="r")
        q_t = rpool.tile([P, D_OUT], F32, tag="q")
        f_t = rpool.tile([P, D_OUT], F32, tag="f")
        rsum = rpool.tile([P, D_IN], F32, tag="rsum")
        tmp = tmppool.tile([P, D_IN, C_OUT, D_OUT], BF16, tag="tmp")
        tmp_flat = tmp[:].rearrange("p a b c -> p (a b c)")

        if not zero_prior:
            nc.scalar.copy(b_t[pxs], b0_sb[pxs])

        for it in range(ITERS):
            first = it == 0
            last = it == ITERS - 1
            uniform0 = first and zero_prior

            # softmax over d (no max-subtraction: logits are O(1) here)
            if not uniform0:
                src = b0_sb if (first and not zero_prior) else b_t
                nc.scalar.activation(c_e[pxs], src[pxs], AF.Exp)
                nc.vector.reduce_sum(rsum[pxs], c_e[pxs],
                                     axis=mybir.AxisListType.X)
                nc.vector.reciprocal(rsum[pxs], rsum[pxs])
                nc.vector.tensor_mul(
                    c_t[pxs], c_e[pxs],
                    rsum[pxs].unsqueeze(2).broadcast_to((npx, D_IN, D_OUT)))

            # s[c,d] = sum_D c[D,d] * u[D,c,d]
            s_scale = 1.0
            if uniform0:
                s_cur = s0_sb
                if not mm_s0:
                    # tile 0: tree-sum u over D on the vector engine; defer
                    # the 1/32 mean scale into the squash (free in ACT args)
                    u_flat = u_t[:].rearrange("p a b c -> p (a b c)")
                    h = D_IN * CD // 2
                    nc.vector.tensor_add(tmp_flat[pxs, 0:h],
                                         u_flat[pxs, 0:h],
                                         u_flat[pxs, h:2 * h])
                    _tree_reduce_flat(nc, tmp_flat, pxs, h, s0_sb[pxs])
                    s_scale = 1.0 / D_IN
            else:
                s_cur = s_t
                nc.vector.tensor_mul(
                    tmp[pxs], u_t[pxs],
                    c_t[pxs].unsqueeze(2)
                    .broadcast_to((npx, D_IN, C_OUT, D_OUT)))
                _tree_reduce_flat(nc, tmp_flat, pxs, D_IN * CD, s_t[pxs])

            if last:
                break

            # squash over c: v = s * n2 / ((1+n2) * sqrt(n2+eps)); when
            # s_cur holds 32*s, fold the 1/32 into the Square scale and the
            # 32 into q so v = s_cur * f comes out right.
            nc.scalar.activation(sq_t[pxs], s_cur[pxs], AF.Square,
                                 scale=s_scale)
            nc.vector.reduce_sum(n2_t[pxs], sq_t[pxs].transpose([0, 2, 1]),
                                 axis=mybir.AxisListType.X)
            nc.scalar.activation(r_t[pxs], n2_t[pxs], AF.Sqrt, bias=EPS)
            if s_scale == 1.0:
                nc.scalar.add(q_t[pxs], n2_t[pxs], 1.0)
            else:
                nc.scalar.activation(q_t[pxs], n2_t[pxs], AF.Identity,
                                     bias=float(D_IN), scale=float(D_IN))
            nc.vector.tensor_mul(f_t[pxs], q_t[pxs], r_t[pxs])
            nc.vector.reciprocal(f_t[pxs], f_t[pxs])
            nc.vector.tensor_mul(f_t[pxs], f_t[pxs], n2_t[pxs])
            nc.vector.tensor_mul(
                v_t[pxs], s_cur[pxs],
                f_t[pxs].unsqueeze(1).broadcast_to((npx, C_OUT, D_OUT)))

            # b[D,d] += sum_c u[D,c,d] * v[c,d]
            nc.vector.tensor_mul(
                tmp[pxs], u_t[pxs],
                v_t[pxs].unsqueeze(1)
                .broadcast_to((npx, D_IN, C_OUT, D_OUT)))
            if uniform0:
                # b was zero: write the reduction straight into b
                _tree_reduce_c(nc, tmp, pxs, b_t[pxs])
            else:
                _tree_reduce_c(nc, tmp, pxs, ak_t[pxs])
                nc.vector.tensor_add(b_t[pxs], b_t[pxs], ak_t[pxs])

        # ---- write s out as [(c,d), px]: PE transpose in 128-row blocks
        s_flat = s_t[:].rearrange("p a b -> p (a b)")
        for blk in range(CD // P):
            pt = psum_t.tile([P, 120], BF16, tag="pt")
            nc.tensor.transpose(
                pt[:, pxs], s_flat[pxs, blk * P:(blk + 1) * P],
                ident[pxs, pxs])
            ob = opool.tile([P, 120], F32, tag="ob")
            nc.scalar.copy(ob[:, pxs], pt[:, pxs])
            nc.sync.dma_start(
                out[blk * P:(blk + 1) * P, r0 * WO:r0 * WO + npx],
                ob[:, pxs])


_CACHE = {}


def _build(zero_prior: bool):
    key = ("v4", zero_prior)
    if key in _CACHE:
        return _CACHE[key]
    nc = bacc.Bacc("TRN2", target_bir_lowering=False, debug=False,
                   enable_asserts=True, num_devices=B)
    xb = nc.dram_tensor("xb", [C_IN, D_IN, H, W], BF16,
                        kind="ExternalInput").ap()
    wt = nc.dram_tensor("wt", [KDIM, D_IN * CD], BF16,
                        kind="ExternalInput").ap()
    b0 = nc.dram_tensor("b0", [P, D_IN, D_OUT], BF16,
                        kind="ExternalInput").ap()
    out = nc.dram_tensor("out", [CD, NPX], F32, kind="ExternalOutput").ap()
    with tile.TileContext(nc) as tc:
        with ExitStack() as ctx:
            _body(ctx, tc, xb, wt, b0, out, zero_prior)
    nc.compile()
    _CACHE[key] = nc
    return nc


def _prep_inputs(x, conv_w, prior):
    # weights: rows (D,c,d) x (C,kh,kw) -> [k=(kh,kw,C), (D,c,d)]
    wt = conv_w.reshape(D_IN, C_OUT, D_OUT, C_IN, KS, KS)
    wt = np.ascontiguousarray(wt.transpose(4, 5, 3, 0, 1, 2)).reshape(KDIM, D_IN * CD)
    wt = wt.astype(ml_dtypes.bfloat16)
    pb = np.broadcast_to(prior.reshape(D_IN, D_OUT), (P, D_IN, D_OUT))
    b0 = np.ascontiguousarray(pb).astype(ml_dtypes.bfloat16)
    xbf = x.astype(ml_dtypes.bfloat16)
    in_maps = [
        {"xb": np.ascontiguousarray(xbf[b]), "wt": wt, "b0": b0}
        for b in range(B)
    ]
    return in_maps


def kernel(x, conv_w, prior):
    x = np.asarray(x, dtype=np.float32)
    conv_w = np.asarray(conv_w, dtype=np.float32)
    prior = np.asarray(prior, dtype=np.float32)
    zero_prior = not np.any(prior)
    nc = _build(zero_prior)
    in_maps = _prep_inputs(x, conv_w, prior)
    res = run_bass_kernel_spmd(nc, in_maps, list(range(B)))
    outs = [res.results[b]["out"].reshape(C_OUT, D_OUT, HO, WO)
            for b in range(B)]
    return np.stack(outs, axis=0).astype(np.float32)


# revision 18
# speedup vs baseline: 3.0868x; 1.0380x over previous
"""ConvCaps dynamic-routing kernel for 8 TRN2 NeuronCores.

Strategy (data-parallel over batch B=8, one batch element per core):
  - Everything in bf16 (tolerance is 2e-2; bf16 lands ~1e-3).
  - Grouped 3x3 conv (groups=D=32) as one bf16 matmul per group per
    pixel tile: stationary = im2col patches [72, npx], moving = weights
    [72, 512], PSUM fp32 -> u tile in SBUF as bf16 [px, D, c, d].
  - iter-0 s (zero prior => uniform c) comes free from TensorE: a second
    moving pass per group accumulates sum_D u into one PSUM bank.
  - Routing einsum contractions run on the Vector engine as bf16
    tensor_tensor multiplies (2x mode) + in-place binary-tree adds
    (2x mode) instead of TENSOR_REDUCE (1x, ~1.6 cyc/elem measured).
  - Output s [px, (c,d)] is PE-transposed to [(c,d), px] and DMA'd out.
"""

import numpy as np
from contextlib import ExitStack

import ml_dtypes

import concourse.bacc as bacc
import concourse.bass as bass
import concourse.tile as tile
import concourse.mybir as mybir
from concourse.bass_utils import run_bass_kernel_spmd
from concourse.masks import make_identity

F32 = mybir.dt.float32
BF16 = mybir.dt.bfloat16
AF = mybir.ActivationFunctionType

B = 8
C_IN, D_IN = 8, 32
C_OUT, D_OUT = 16, 32
KS = 3
H = W = 32
HO = WO = 30
NPX = HO * WO                 # 900 output pixels per batch element
KDIM = C_IN * KS * KS         # 72 = contraction dim of the conv matmul
CD = C_OUT * D_OUT            # 512 out-channels per group
ITERS = 3
P = 128
EPS = 1e-8
ROW_TILES = [(0, 4), (4, 4), (8, 4), (12, 4), (16, 4), (20, 4), (24, 4), (28, 2)]


def _tree_reduce_flat(nc, tmp, pxs, n, dst):
    """In-place binary tree sum of tmp[pxs, :n] down to dst[pxs] (n//? -> 512).

    tmp is a [P, n] view; halves n each level with bf16 tensor_tensor adds
    (2x DVE mode) until 1024, then the final add writes into dst (512 wide).
    """
    while n > 1024:
        h = n // 2
        nc.vector.tensor_add(tmp[pxs, 0:h], tmp[pxs, 0:h], tmp[pxs, h:n])
        n = h
    nc.vector.tensor_add(dst, tmp[pxs, 0:512], tmp[pxs, 512:1024])


def _tree_reduce_c(nc, tmp, pxs, dst):
    """Sum tmp[pxs, D, c, d] over c (16) -> dst [pxs, D, d] (in-place tree)."""
    c = C_OUT
    while c > 2:
        h = c // 2
        nc.vector.tensor_add(tmp[pxs, :, 0:h, :], tmp[pxs, :, 0:h, :],
                             tmp[pxs, :, h:c, :])
        c = h
    nc.vector.tensor_add(dst, tmp[pxs, :, 0, :], tmp[pxs, :, 1, :])


def _body(ctx, tc, xb, wt, b0, out, zero_prior):
    nc = tc.nc
    consts = ctx.enter_context(tc.tile_pool(name="consts", bufs=1))
    wpool = ctx.enter_context(tc.tile_pool(name="wpool", bufs=1))
    x9pool = ctx.enter_context(tc.tile_pool(name="x9pool", bufs=2))
    upool = ctx.enter_context(tc.tile_pool(name="upool", bufs=2))
    s0pool = ctx.enter_context(tc.tile_pool(name="s0pool", bufs=2))
    rpool = ctx.enter_context(tc.tile_pool(name="rpool", bufs=2))
    tmppool = ctx.enter_context(tc.tile_pool(name="tmppool", bufs=2))
    opool = ctx.enter_context(tc.tile_pool(name="opool", bufs=2))
    psum_c = ctx.enter_context(tc.tile_pool(name="psum_c", bufs=3, space="PSUM"))
    psum_s = ctx.enter_context(tc.tile_pool(name="psum_s", bufs=2, space="PSUM"))
    psum_t = ctx.enter_context(tc.tile_pool(name="psum_t", bufs=2, space="PSUM"))

    ident = consts.tile([P, P], BF16)
    make_identity(nc, ident)
    w_sb = wpool.tile([KDIM, D_IN * CD], BF16)
    nc.scalar.dma_start(w_sb[:], wt)
    # warm the PE clock (HAM releases the throttle after ~3.4us of
    # sustained activity) while the first im2col DMAs are in flight
    for i in range(10):
        pw = psum_s.tile([P, P], BF16, tag="warm", bufs=1)
        nc.tensor.transpose(pw[:], ident[:], ident[:])
    for cval in (EPS, 1.0, 32.0):
        cb = consts.tile([P, 1], F32, tag=f"const_{cval}")
        nc.gpsimd.memset(cb[:], cval)
        nc.const_aps.aps[(F32, cval)] = cb[:]
    if not zero_prior:
        b0_sb = consts.tile([P, D_IN, D_OUT], BF16)
        nc.sync.dma_start(b0_sb[:], b0)

    for ti, (r0, nr) in enumerate(ROW_TILES):
        npx = nr * WO
        pxs = slice(0, npx)
        # tile 0 computes s0 with a vector tree instead of the extra PE
        # pass: u is on the critical path at startup, nothing overlaps it.
        mm_s0 = zero_prior and ti > 0

        # ---- im2col: 9 shifted window loads; partition k = (kh*3+kw)*8 + C
        # x is [C, H, W, D] on the host so each (kh,kw) window is a dense
        # (w,d) run -> one DMA per k-position with ~2KB packets.
        x9 = x9pool.tile([KDIM, 4, WO, D_IN], BF16, tag="x9")
        di = 0
        for kh in range(KS):
            for kw in range(KS):
                kk = kh * KS + kw
                eng = nc.scalar if (ti == 0 and di % 2) else nc.sync
                eng.dma_start(
                    x9[kk * C_IN:(kk + 1) * C_IN, 0:nr, :, :],
                    xb[:, r0 + kh:r0 + kh + nr, kw:kw + WO, :],
                )
                di += 1

        # ---- grouped conv: one bf16 matmul per group; a second moving pass
        # accumulates sum_D u into ps0 (free iter-0 s when prior is zero).
        u_t = upool.tile([P, D_IN, C_OUT, D_OUT], BF16, tag="u")
        s0_sb = s0pool.tile([P, C_OUT, D_OUT], BF16, tag="s0")
        if mm_s0:
            ps0 = psum_s.tile([P, CD], F32, tag="ps0")
        for g in range(D_IN):
            pu = psum_c.tile([P, CD], F32, tag="pu")
            nc.tensor.matmul(
                pu[pxs, :],
                x9[:, 0:nr, :, g],
                w_sb[:, g * CD:(g + 1) * CD],
                start=True, stop=True,
            )
            if mm_s0:
                nc.tensor.matmul(
                    ps0[pxs, :],
                    x9[:, 0:nr, :, g],
                    w_sb[:, g * CD:(g + 1) * CD],
                    start=(g == 0), stop=(g == D_IN - 1),
                    skip_group_check=True,
                )
            nc.scalar.copy(u_t[pxs, g], pu[pxs, :])
        if mm_s0:
            nc.scalar.activation(s0_sb[pxs], ps0[pxs, :], AF.Copy,
                                 scale=1.0 / D_IN)

        # ---- routing state tiles
        b_t = rpool.tile([P, D_IN, D_OUT], BF16, tag="b")
        c_e = rpool.tile([P, D_IN, D_OUT], BF16, tag="ce")
        c_t = rpool.tile([P, D_IN, D_OUT], BF16, tag="c")
        ak_t = rpool.tile([P, D_IN, D_OUT], BF16, tag="ak")
        s_t = rpool.tile([P, C_OUT, D_OUT], BF16, tag="s")
        sq_t = rpool.tile([P, C_OUT, D_OUT], F32, tag="sq")
        v_t = rpool.tile([P, C_OUT, D_OUT], BF16, tag="v")
        n2_t = rpool.tile([P, D_OUT], F32, tag="n2")
        r_t = rpool.tile([P, D_OUT], F32, tag="r")
        q_t = rpool.tile([P, D_OUT], F32, tag="q")
        f_t = rpool.tile([P, D_OUT], F32, tag="f")
        rsum = rpool.tile([P, D_IN], F32, tag="rsum")
        tmp = tmppool.tile([P, D_IN, C_OUT, D_OUT], BF16, tag="tmp")
        tmp_flat = tmp[:].rearrange("p a b c -> p (a b c)")

        if not zero_prior:
            nc.scalar.copy(b_t[pxs], b0_sb[pxs])

        for it in range(ITERS):
            first = it == 0
            last = it == ITERS - 1
            uniform0 = first and zero_prior

            # softmax over d (no max-subtraction: logits are O(1) here)
            if not uniform0:
                src = b0_sb if (first and not zero_prior) else b_t
                nc.scalar.activation(c_e[pxs], src[pxs], AF.Exp)
                nc.vector.reduce_sum(rsum[pxs], c_e[pxs],
                                     axis=mybir.AxisListType.X)
                nc.vector.reciprocal(rsum[pxs], rsum[pxs])
                nc.vector.tensor_mul(
                    c_t[pxs], c_e[pxs],
                    rsum[pxs].unsqueeze(2).broadcast_to((npx, D_IN, D_OUT)))

            # s[c,d] = sum_D c[D,d] * u[D,c,d]
            s_scale = 1.0
            if uniform0:
                s_cur = s0_sb
                if not mm_s0:
                    # tile 0: tree-sum u over D on the vector engine; defer
                    # the 1/32 mean scale into the squash (free in ACT args)
                    u_flat = u_t[:].rearrange("p a b c -> p (a b c)")
                    h = D_IN * CD // 2
                    nc.vector.tensor_add(tmp_flat[pxs, 0:h],
                                         u_flat[pxs, 0:h],
                                         u_flat[pxs, h:2 * h])
                    _tree_reduce_flat(nc, tmp_flat, pxs, h, s0_sb[pxs])
                    s_scale = 1.0 / D_IN
            else:
                s_cur = s_t
                nc.vector.tensor_mul(
                    tmp[pxs], u_t[pxs],
                    c_t[pxs].unsqueeze(2)
                    .broadcast_to((npx, D_IN, C_OUT, D_OUT)))
                _tree_reduce_flat(nc, tmp_flat, pxs, D_IN * CD, s_t[pxs])

            if last:
                break

            # squash over c: v = s * n2 / ((1+n2) * sqrt(n2+eps)); when
            # s_cur holds 32*s, fold the 1/32 into the Square scale and the
            # 32 into q so v = s_cur * f comes out right.
            nc.scalar.activation(sq_t[pxs], s_cur[pxs], AF.Square,
                                 scale=s_scale)
            nc.vector.reduce_sum(n2_t[pxs], sq_t[pxs].transpose([0, 2, 1]),
                                 axis=mybir.AxisListType.X)
            nc.scalar.activation(r_t[pxs], n2_t[pxs], AF.Sqrt, bias=EPS)
            if s_scale == 1.0:
                nc.scalar.add(q_t[pxs], n2_t[pxs], 1.0)
            else:
                nc.scalar.activation(q_t[pxs], n2_t[pxs], AF.Identity,
                                     bias=float(D_IN), scale=float(D_IN))
            nc.vector.tensor_mul(f_t[pxs], q_t[pxs], r_t[pxs])
            nc.vector.reciprocal(f_t[pxs], f_t[pxs])
            nc.vector.tensor_mul(f_t[pxs], f_t[pxs], n2_t[pxs])
            nc.vector.tensor_mul(
                v_t[pxs], s_cur[pxs],
                f_t[pxs].unsqueeze(1).broadcast_to((npx, C_OUT, D_OUT)))

            # b[D,d] += sum_c u[D,c,d] * v[c,d]
            nc.vector.tensor_mul(
                tmp[pxs], u_t[pxs],
                v_t[pxs].unsqueeze(1)
                .broadcast_to((npx, D_IN, C_OUT, D_OUT)))
            if uniform0:
                # b was zero: write the reduction straight into b
                _tree_reduce_c(nc, tmp, pxs, b_t[pxs])
            else:
                _tree_reduce_c(nc, tmp, pxs, ak_t[pxs])
                nc.vector.tensor_add(b_t[pxs], b_t[pxs], ak_t[pxs])

        # ---- write s out as [(c,d), px]: PE transpose in 128-row blocks
        s_flat = s_t[:].rearrange("p a b -> p (a b)")
        for blk in range(CD // P):
            pt = psum_t.tile([P, 120], BF16, tag="pt")
            nc.tensor.transpose(
                pt[:, pxs], s_flat[pxs, blk * P:(blk + 1) * P],
                ident[pxs, pxs])
            ob = opool.tile([P, 120], F32, tag="ob")
            nc.scalar.copy(ob[:, pxs], pt[:, pxs])
            nc.sync.dma_start(
                out[blk * P:(blk + 1) * P, r0 * WO:r0 * WO + npx],
                ob[:, pxs])


_CACHE = {}


def _build(zero_prior: bool):
    key = ("v4", zero_prior)
    if key in _CACHE:
        return _CACHE[key]
    nc = bacc.Bacc("TRN2", target_bir_lowering=False, debug=False,
                   enable_asserts=True, num_devices=B)
    xb = nc.dram_tensor("xb", [C_IN, H, W, D_IN], BF16,
                        kind="ExternalInput").ap()
    wt = nc.dram_tensor("wt", [KDIM, D_IN * CD], BF16,
                        kind="ExternalInput").ap()
    b0 = nc.dram_tensor("b0", [P, D_IN, D_OUT], BF16,
                        kind="ExternalInput").ap()
    out = nc.dram_tensor("out", [CD, NPX], F32, kind="ExternalOutput").ap()
    with tile.TileContext(nc) as tc:
        with ExitStack() as ctx:
            _body(ctx, tc, xb, wt, b0, out, zero_prior)
    nc.compile()
    _CACHE[key] = nc
    return nc


def _prep_inputs(x, conv_w, prior):
    # weights: rows (D,c,d) x (C,kh,kw) -> [k=(kh,kw,C), (D,c,d)]
    wt = conv_w.reshape(D_IN, C_OUT, D_OUT, C_IN, KS, KS)
    wt = np.ascontiguousarray(wt.transpose(4, 5, 3, 0, 1, 2)).reshape(KDIM, D_IN * CD)
    wt = wt.astype(ml_dtypes.bfloat16)
    pb = np.broadcast_to(prior.reshape(D_IN, D_OUT), (P, D_IN, D_OUT))
    b0 = np.ascontiguousarray(pb).astype(ml_dtypes.bfloat16)
    # [B, C, D, H, W] -> [B, C, H, W, D] so im2col windows are dense runs
    xbf = np.ascontiguousarray(x.transpose(0, 1, 3, 4, 2)).astype(
        ml_dtypes.bfloat16)
    in_maps = [
        {"xb": xbf[b], "wt": wt, "b0": b0}
        for b in range(B)
    ]
    return in_maps


def kernel(x, conv_w, prior):
    x = np.asarray(x, dtype=np.float32)
    conv_w = np.asarray(conv_w, dtype=np.float32)
    prior = np.asarray(prior, dtype=np.float32)
    zero_prior = not np.any(prior)
    nc = _build(zero_prior)
    in_maps = _prep_inputs(x, conv_w, prior)
    res = run_bass_kernel_spmd(nc, in_maps, list(range(B)))
    outs = [res.results[b]["out"].reshape(C_OUT, D_OUT, HO, WO)
            for b in range(B)]
    return np.stack(outs, axis=0).astype(np.float32)


# revision 22
# speedup vs baseline: 3.1385x; 1.0168x over previous
"""ConvCaps dynamic-routing kernel for 8 TRN2 NeuronCores.

Strategy (data-parallel over batch B=8, one batch element per core):
  - Everything in bf16 (tolerance is 2e-2; bf16 lands ~1e-3).
  - Grouped 3x3 conv (groups=D=32) as one bf16 matmul per group per
    pixel tile: stationary = im2col patches [72, npx], moving = weights
    [72, 512], PSUM fp32 -> u tile in SBUF as bf16 [px, D, c, d].
  - iter-0 s (zero prior => uniform c) comes free from TensorE: a second
    moving pass per group accumulates sum_D u into one PSUM bank.
  - Routing einsum contractions run on the Vector engine as bf16
    tensor_tensor multiplies (2x mode) + in-place binary-tree adds
    (2x mode) instead of TENSOR_REDUCE (1x, ~1.6 cyc/elem measured).
  - Output s [px, (c,d)] is PE-transposed to [(c,d), px] and DMA'd out.
"""

import numpy as np
from contextlib import ExitStack

import ml_dtypes

import concourse.bacc as bacc
import concourse.bass as bass
import concourse.tile as tile
import concourse.mybir as mybir
from concourse.bass_utils import run_bass_kernel_spmd
from concourse.masks import make_identity

F32 = mybir.dt.float32
BF16 = mybir.dt.bfloat16
AF = mybir.ActivationFunctionType

B = 8
C_IN, D_IN = 8, 32
C_OUT, D_OUT = 16, 32
KS = 3
H = W = 32
HO = WO = 30
NPX = HO * WO                 # 900 output pixels per batch element
KDIM = C_IN * KS * KS         # 72 = contraction dim of the conv matmul
CD = C_OUT * D_OUT            # 512 out-channels per group
ITERS = 3
P = 128
EPS = 1e-8
ROW_TILES = [(0, 4), (4, 4), (8, 4), (12, 4), (16, 4), (20, 4), (24, 4), (28, 2)]


def _tree_reduce_flat(nc, tmp, pxs, n, dst):
    """In-place binary tree sum of tmp[pxs, :n] down to dst[pxs] (n//? -> 512).

    tmp is a [P, n] view; halves n each level with bf16 tensor_tensor adds
    (2x DVE mode) until 1024, then the final add writes into dst (512 wide).
    """
    while n > 1024:
        h = n // 2
        nc.vector.tensor_add(tmp[pxs, 0:h], tmp[pxs, 0:h], tmp[pxs, h:n])
        n = h
    nc.vector.tensor_add(dst, tmp[pxs, 0:512], tmp[pxs, 512:1024])


def _tree_reduce_c(nc, tmp, pxs, dst):
    """Sum tmp[pxs, D, c, d] over c (16) -> dst [pxs, D, d] (in-place tree)."""
    c = C_OUT
    while c > 2:
        h = c // 2
        nc.vector.tensor_add(tmp[pxs, :, 0:h, :], tmp[pxs, :, 0:h, :],
                             tmp[pxs, :, h:c, :])
        c = h
    nc.vector.tensor_add(dst, tmp[pxs, :, 0, :], tmp[pxs, :, 1, :])


def _body(ctx, tc, xb, wt, b0, out, zero_prior):
    nc = tc.nc
    consts = ctx.enter_context(tc.tile_pool(name="consts", bufs=1))
    wpool = ctx.enter_context(tc.tile_pool(name="wpool", bufs=1))
    x9pool = ctx.enter_context(tc.tile_pool(name="x9pool", bufs=2))
    upool = ctx.enter_context(tc.tile_pool(name="upool", bufs=2))
    s0pool = ctx.enter_context(tc.tile_pool(name="s0pool", bufs=2))
    rpool = ctx.enter_context(tc.tile_pool(name="rpool", bufs=2))
    tmppool = ctx.enter_context(tc.tile_pool(name="tmppool", bufs=2))
    opool = ctx.enter_context(tc.tile_pool(name="opool", bufs=2))
    psum_c = ctx.enter_context(tc.tile_pool(name="psum_c", bufs=4, space="PSUM"))
    psum_s = ctx.enter_context(tc.tile_pool(name="psum_s", bufs=1, space="PSUM"))
    psum_t = ctx.enter_context(tc.tile_pool(name="psum_t", bufs=2, space="PSUM"))

    ident = consts.tile([P, P], BF16)
    make_identity(nc, ident)
    w_sb = wpool.tile([KDIM, D_IN * CD], BF16)
    nc.scalar.dma_start(w_sb[:], wt)
    # warm the PE clock (HAM releases the throttle after ~3.4us of
    # sustained activity) while the first im2col DMAs are in flight
    for i in range(10):
        pw = psum_s.tile([P, P], BF16, tag="warm", bufs=1)
        nc.tensor.transpose(pw[:], ident[:], ident[:])
    for cval in (EPS, 1.0, 32.0):
        cb = consts.tile([P, 1], F32, tag=f"const_{cval}")
        nc.gpsimd.memset(cb[:], cval)
        nc.const_aps.aps[(F32, cval)] = cb[:]
    if not zero_prior:
        b0_sb = consts.tile([P, D_IN, D_OUT], BF16)
        nc.sync.dma_start(b0_sb[:], b0)

    for ti, (r0, nr) in enumerate(ROW_TILES):
        npx = nr * WO
        pxs = slice(0, npx)
        # tile 0 computes s0 with a vector tree instead of the extra PE
        # pass: u is on the critical path at startup, nothing overlaps it.
        mm_s0 = zero_prior and ti > 0

        # ---- im2col: 9 shifted window loads; partition k = (kh*3+kw)*8 + C
        # x is [C, H, W, D] on the host so each (kh,kw) window is a dense
        # (w,d) run -> one DMA per k-position with ~2KB packets.
        x9 = x9pool.tile([KDIM, 4, WO, D_IN], BF16, tag="x9")
        di = 0
        for kh in range(KS):
            for kw in range(KS):
                kk = kh * KS + kw
                eng = nc.scalar if (ti == 0 and di % 2) else nc.sync
                eng.dma_start(
                    x9[kk * C_IN:(kk + 1) * C_IN, 0:nr, :, :],
                    xb[:, r0 + kh:r0 + kh + nr, kw:kw + WO, :],
                )
                di += 1

        # ---- grouped conv: one bf16 matmul per group; a second moving pass
        # accumulates sum_D u into ps0 (free iter-0 s when prior is zero).
        u_t = upool.tile([P, D_IN, C_OUT, D_OUT], BF16, tag="u")
        s0_sb = s0pool.tile([P, C_OUT, D_OUT], BF16, tag="s0")
        if mm_s0:
            ps0 = psum_s.tile([P, CD], F32, tag="ps0")
        for g in range(D_IN):
            pu = psum_c.tile([P, CD], F32, tag="pu")
            nc.tensor.matmul(
                pu[pxs, :],
                x9[:, 0:nr, :, g],
                w_sb[:, g * CD:(g + 1) * CD],
                start=True, stop=True,
            )
            if mm_s0:
                nc.tensor.matmul(
                    ps0[pxs, :],
                    x9[:, 0:nr, :, g],
                    w_sb[:, g * CD:(g + 1) * CD],
                    start=(g == 0), stop=(g == D_IN - 1),
                    skip_group_check=True,
                )
            if ti == 0 and g % 2:
                # startup: vector is idle, let it share the PSUM drain
                nc.vector.tensor_copy(u_t[pxs, g], pu[pxs, :])
            else:
                nc.scalar.copy(u_t[pxs, g], pu[pxs, :])
        if mm_s0:
            nc.scalar.activation(s0_sb[pxs], ps0[pxs, :], AF.Copy,
                                 scale=1.0 / D_IN)

        # ---- routing state tiles
        b_t = rpool.tile([P, D_IN, D_OUT], BF16, tag="b")
        c_e = rpool.tile([P, D_IN, D_OUT], BF16, tag="ce")
        c_t = rpool.tile([P, D_IN, D_OUT], BF16, tag="c")
        ak_t = rpool.tile([P, D_IN, D_OUT], BF16, tag="ak")
        s_t = rpool.tile([P, C_OUT, D_OUT], BF16, tag="s")
        sq_t = rpool.tile([P, C_OUT, D_OUT], F32, tag="sq")
        v_t = rpool.tile([P, C_OUT, D_OUT], BF16, tag="v")
        n2_t = rpool.tile([P, D_OUT], F32, tag="n2")
        r_t = rpool.tile([P, D_OUT], F32, tag="r")
        q_t = rpool.tile([P, D_OUT], F32, tag="q")
        f_t = rpool.tile([P, D_OUT], F32, tag="f")
        rsum = rpool.tile([P, D_IN], F32, tag="rsum")
        tmp = tmppool.tile([P, D_IN, C_OUT, D_OUT], BF16, tag="tmp")
        tmp_flat = tmp[:].rearrange("p a b c -> p (a b c)")

        if not zero_prior:
            nc.scalar.copy(b_t[pxs], b0_sb[pxs])

        for it in range(ITERS):
            first = it == 0
            last = it == ITERS - 1
            uniform0 = first and zero_prior

            # softmax over d (no max-subtraction: logits are O(1) here)
            if not uniform0:
                src = b0_sb if (first and not zero_prior) else b_t
                nc.scalar.activation(c_e[pxs], src[pxs], AF.Exp)
                nc.vector.reduce_sum(rsum[pxs], c_e[pxs],
                                     axis=mybir.AxisListType.X)
                nc.vector.reciprocal(rsum[pxs], rsum[pxs])
                nc.vector.tensor_mul(
                    c_t[pxs], c_e[pxs],
                    rsum[pxs].unsqueeze(2).broadcast_to((npx, D_IN, D_OUT)))

            # s[c,d] = sum_D c[D,d] * u[D,c,d]
            s_scale = 1.0
            if uniform0:
                s_cur = s0_sb
                if not mm_s0:
                    # tile 0: tree-sum u over D on the vector engine; defer
                    # the 1/32 mean scale into the squash (free in ACT args)
                    u_flat = u_t[:].rearrange("p a b c -> p (a b c)")
                    h = D_IN * CD // 2
                    nc.vector.tensor_add(tmp_flat[pxs, 0:h],
                                         u_flat[pxs, 0:h],
                                         u_flat[pxs, h:2 * h])
                    _tree_reduce_flat(nc, tmp_flat, pxs, h, s0_sb[pxs])
                    s_scale = 1.0 / D_IN
            else:
                s_cur = s_t
                nc.vector.tensor_mul(
                    tmp[pxs], u_t[pxs],
                    c_t[pxs].unsqueeze(2)
                    .broadcast_to((npx, D_IN, C_OUT, D_OUT)))
                _tree_reduce_flat(nc, tmp_flat, pxs, D_IN * CD, s_t[pxs])

            if last:
                break

            # squash over c: v = s * n2 / ((1+n2) * sqrt(n2+eps)); when
            # s_cur holds 32*s, fold the 1/32 into the Square scale and the
            # 32 into q so v = s_cur * f comes out right.
            nc.scalar.activation(sq_t[pxs], s_cur[pxs], AF.Square,
                                 scale=s_scale)
            nc.vector.reduce_sum(n2_t[pxs], sq_t[pxs].transpose([0, 2, 1]),
                                 axis=mybir.AxisListType.X)
            nc.scalar.activation(r_t[pxs], n2_t[pxs], AF.Sqrt, bias=EPS)
            if s_scale == 1.0:
                nc.scalar.add(q_t[pxs], n2_t[pxs], 1.0)
            else:
                nc.scalar.activation(q_t[pxs], n2_t[pxs], AF.Identity,
                                     bias=float(D_IN), scale=float(D_IN))
            nc.vector.tensor_mul(f_t[pxs], q_t[pxs], r_t[pxs])
            nc.vector.reciprocal(f_t[pxs], f_t[pxs])
            nc.vector.tensor_mul(f_t[pxs], f_t[pxs], n2_t[pxs])
            nc.vector.tensor_mul(
                v_t[pxs], s_cur[pxs],
                f_t[pxs].unsqueeze(1).broadcast_to((npx, C_OUT, D_OUT)))

            # b[D,d] += sum_c u[D,c,d] * v[c,d]
            nc.vector.tensor_mul(
                tmp[pxs], u_t[pxs],
                v_t[pxs].unsqueeze(1)
                .broadcast_to((npx, D_IN, C_OUT, D_OUT)))
            if uniform0:
                # b was zero: write the reduction straight into b
                _tree_reduce_c(nc, tmp, pxs, b_t[pxs])
            else:
                _tree_reduce_c(nc, tmp, pxs, ak_t[pxs])
                nc.vector.tensor_add(b_t[pxs], b_t[pxs], ak_t[pxs])

        # ---- write s out as [(c,d), px]: PE transpose in 128-row blocks
        s_flat = s_t[:].rearrange("p a b -> p (a b)")
        for blk in range(CD // P):
            pt = psum_t.tile([P, 120], BF16, tag="pt")
            nc.tensor.transpose(
                pt[:, pxs], s_flat[pxs, blk * P:(blk + 1) * P],
                ident[pxs, pxs])
            ob = opool.tile([P, 120], F32, tag="ob")
            nc.scalar.copy(ob[:, pxs], pt[:, pxs])
            nc.sync.dma_start(
                out[blk * P:(blk + 1) * P, r0 * WO:r0 * WO + npx],
                ob[:, pxs])


_CACHE = {}


def _build(zero_prior: bool):
    key = ("v4", zero_prior)
    if key in _CACHE:
        return _CACHE[key]
    nc = bacc.Bacc("TRN2", target_bir_lowering=False, debug=False,
                   enable_asserts=True, num_devices=B)
    xb = nc.dram_tensor("xb", [C_IN, H, W, D_IN], BF16,
                        kind="ExternalInput").ap()
    wt = nc.dram_tensor("wt", [KDIM, D_IN * CD], BF16,
                        kind="ExternalInput").ap()
    b0 = nc.dram_tensor("b0", [P, D_IN, D_OUT], BF16,
                        kind="ExternalInput").ap()
    out = nc.dram_tensor("out", [CD, NPX], F32, kind="ExternalOutput").ap()
    with tile.TileContext(nc) as tc:
        with ExitStack() as ctx:
            _body(ctx, tc, xb, wt, b0, out, zero_prior)
    nc.compile()
    _CACHE[key] = nc
    return nc


def _prep_inputs(x, conv_w, prior):
    # weights: rows (D,c,d) x (C,kh,kw) -> [k=(kh,kw,C), (D,c,d)]
    wt = conv_w.reshape(D_IN, C_OUT, D_OUT, C_IN, KS, KS)
    wt = np.ascontiguousarray(wt.transpose(4, 5, 3, 0, 1, 2)).reshape(KDIM, D_IN * CD)
    wt = wt.astype(ml_dtypes.bfloat16)
    pb = np.broadcast_to(prior.reshape(D_IN, D_OUT), (P, D_IN, D_OUT))
    b0 = np.ascontiguousarray(pb).astype(ml_dtypes.bfloat16)
    # [B, C, D, H, W] -> [B, C, H, W, D] so im2col windows are dense runs
    xbf = np.ascontiguousarray(x.transpose(0, 1, 3, 4, 2)).astype(
        ml_dtypes.bfloat16)
    in_maps = [
        {"xb": xbf[b], "wt": wt, "b0": b0}
        for b in range(B)
    ]
    return in_maps


def kernel(x, conv_w, prior):
    x = np.asarray(x, dtype=np.float32)
    conv_w = np.asarray(conv_w, dtype=np.float32)
    prior = np.asarray(prior, dtype=np.float32)
    zero_prior = not np.any(prior)
    nc = _build(zero_prior)
    in_maps = _prep_inputs(x, conv_w, prior)
    res = run_bass_kernel_spmd(nc, in_maps, list(range(B)))
    outs = [res.results[b]["out"].reshape(C_OUT, D_OUT, HO, WO)
            for b in range(B)]
    return np.stack(outs, axis=0).astype(np.float32)
